# revision 14
# baseline (speedup 1.0000x reference)
"""Attention (B=4, S=4096, W=512, E=64) on 8 TRN2 NeuronCores.

Sharding: core c handles batch b = c//2, query half h = c%2 (2048 queries).
Each core receives x[b]^T as bf16 with the key/value columns ordered so that
this core's query half occupies columns [0, 2048) (softmax over keys is
permutation invariant as long as K and V share the order). K/V are computed
for the full sequence locally; flash-style attention over the core's query
half. No collectives.

Math simplifications vs the reference:
  - K bias bk drops out of softmax entirely (its contribution to the score
    is constant along the key axis only through Q.bk, a per-query constant).
  - V bias bv commutes with the softmax average: Z = softmax(S) V + bv, so
    bv is added on the host during unsharding.
  - Only the Q bias bq remains on-device (fused into the Q projection
    evacuation on ScalarE as an Identity-activation with per-partition bias).

Per-core dataflow (bf16 matmul inputs, fp32 PSUM):
  x^T [512,4096] streamed in 16 HWDGE transfers issued from 4 different
  engines (parallel descriptor generation); ~14 warmup matmuls run during
  the DMA wait so the PE HAM clock-gate is released before real work.
  Projections: kv = V^T (p0:64) / K^T (p64:128), Q^T duplicated on both
  halves; PSUM evacuation on ScalarE (frees VectorE for exp work).
  K^T replicated to partitions 0:64 via SBUF->SBUF DMA (score row-group 0).
  Attention in two passes over query-chunk pairs; per key-tile pair kp:
    scores S^T for two chunks, two k-tiles packed in PE row groups (e=64),
    matmuls grouped by stationary operand;
    exp: ScalarE does chunk A exactly (ACT table), VectorE does chunk B via
    a Schraudolph bit-trick (one tensor_scalar mul-add fp32->int16, the
    int16 buffer reinterpreted as bf16 ~= exp with ~3% max error, which
    softmax normalization cancels to ~1e-3 in Z);
    AV accumulated into per-chunk PSUM banks with a trailing ones column
    in V' producing the softmax denominator.
  Normalize: PE-transpose Z'^T, reciprocal + scale on VectorE, DMA out.
"""

import numpy as np
import ml_dtypes

import concourse.bass as bass
import concourse.mybir as mybir
import concourse.tile as tile
from concourse import bacc
from concourse.bass import ts
from concourse.masks import make_identity
from concourse.bass_utils import run_bass_kernel_spmd

BF16 = mybir.dt.bfloat16
F32 = mybir.dt.float32
I16 = mybir.dt.int16
NP_BF16 = ml_dtypes.bfloat16

B = 4
S_FULL = 4096
W = 512
E = 64
TQ = 2048  # queries per core
WT = W // 128  # 4 contraction tiles
KT = S_FULL // 128  # 32 key tiles
KP = KT // 2  # 16 key-tile pairs
NCH = S_FULL // 512  # 8 projection chunks
QC = TQ // 512  # 4 query chunks of 512
SCALE = 0.125  # 1/sqrt(E)
N_WARMUP = 14

# Schraudolph exp-approximation constants for bf16 bit-trick on VectorE:
# bits16(exp(SCALE*s)) ~= round(A*s + Bc); bf16 = 8 exp bits, 7 mantissa.
A_SCH = float(SCALE * np.log2(np.e) * 128.0)
B_SCH = float(127.0 * 128.0 - 5.8)

_NC_CACHE = {}


def build_nc():
    nc = bacc.Bacc("TRN2", target_bir_lowering=False)
    xT = nc.dram_tensor("xT", [W, S_FULL], BF16, kind="ExternalInput")
    wqq = nc.dram_tensor("wqq", [W, 128], BF16, kind="ExternalInput")
    wkv = nc.dram_tensor("wkv", [W, 128], BF16, kind="ExternalInput")
    bqq = nc.dram_tensor("bqq", [128, 1], F32, kind="ExternalInput")
    y = nc.dram_tensor("y", [TQ, E], F32, kind="ExternalOutput")

    MULT = mybir.AluOpType.mult
    ADD = mybir.AluOpType.add
    EXP = mybir.ActivationFunctionType.Exp

    with tile.TileContext(nc) as tc:
        with (
            tc.tile_pool(name="const", bufs=1) as const,
            tc.tile_pool(name="psA", bufs=2, space="PSUM") as psA,
            tc.tile_pool(name="psS", bufs=2, space="PSUM") as psS,
            tc.tile_pool(name="psZ", bufs=2, space="PSUM") as psZ,
            tc.tile_pool(name="pp0", bufs=2) as pp0,
            tc.tile_pool(name="pp1", bufs=2) as pp1,
            tc.tile_pool(name="zsb", bufs=2) as zsbp,
            tc.tile_pool(name="small", bufs=2) as small,
            tc.tile_pool(name="outp", bufs=2) as outp,
        ):
            # weights/bias first so warmup matmuls can start early
            wkv_sb = const.tile([128, WT, 128], BF16)
            wqq_sb = const.tile([128, WT, 128], BF16)
            nc.scalar.dma_start(
                out=wkv_sb, in_=wkv[:, :].rearrange("(t p) m -> p t m", t=WT)
            )
            nc.scalar.dma_start(
                out=wqq_sb, in_=wqq[:, :].rearrange("(t p) m -> p t m", t=WT)
            )
            bqq_sb = const.tile([128, 1], F32)
            nc.scalar.dma_start(out=bqq_sb, in_=bqq[:, :])

            # x^T streamed in; descriptor generation spread over 4 engines
            xt_sb = const.tile([128, WT, S_FULL], BF16)
            dma_engines = [nc.sync, nc.scalar]
            for ch2 in range(NCH // 2):
                for t in range(WT):
                    eng = dma_engines[(ch2 * WT + t) % 2]
                    eng.dma_start(
                        out=xt_sb[:, t, ts(ch2, 1024)],
                        in_=xT[t * 128:(t + 1) * 128, ts(ch2, 1024)],
                    )

            ident_bf = const.tile([64, 64], BF16)
            make_identity(nc, ident_bf)
            ident_f32 = const.tile([E + 1, E + 1], F32)
            make_identity(nc, ident_f32)

            kv_sb = const.tile([128, S_FULL], BF16)  # V^T (p0:64) / K^T (p64:)
            krep = const.tile([64, S_FULL], BF16)  # K^T replica on p0:64
            qtpair = const.tile([128, TQ], BF16)  # Q^T on both halves
            vp_sb = const.tile([128, KT, E + 1], BF16)  # V' = [V | 1]
            nc.gpsimd.memset(vp_sb, 1.0)

            # warmup matmuls: junk results, keep PE busy through the HAM
            # cold window while the x stream is in flight
            wu = psA.tile([128, 512], F32, tag="mm", name="wu")
            for i in range(N_WARMUP):
                nc.tensor.matmul(
                    wu, wkv_sb[:, i % WT, :], wqq_sb[:, :, :],
                    start=True, stop=True,
                )

            def emit_kv(ch):
                ps = psA.tile([128, 512], F32, tag="mm", name=f"pskv{ch}")
                for t in range(WT):
                    nc.tensor.matmul(
                        ps,
                        wkv_sb[:, t, :],
                        xt_sb[:, t, ts(ch, 512)],
                        start=(t == 0),
                        stop=(t == WT - 1),
                    )
                nc.scalar.copy(kv_sb[:, ts(ch, 512)], ps)
                nc.sync.dma_start(
                    out=krep[:, ts(ch, 512)], in_=kv_sb[64:128, ts(ch, 512)]
                )
                # V^T -> V' tiles (4 PE transposes, one batched DVE copy)
                vt = psA.tile([128, 4, E], BF16, tag="mm", name=f"vt{ch}")
                for j in range(4):
                    nc.tensor.transpose(
                        vt[:, j, :], kv_sb[0:64, ts(4 * ch + j, 128)], ident_bf
                    )
                nc.vector.tensor_copy(vp_sb[:, 4 * ch:4 * ch + 4, 0:E], vt)

            def emit_q(ch):
                ps = psA.tile([128, 512], F32, tag="mm", name=f"psq{ch}")
                for t in range(WT):
                    nc.tensor.matmul(
                        ps,
                        wqq_sb[:, t, :],
                        xt_sb[:, t, ts(ch, 512)],
                        start=(t == 0),
                        stop=(t == WT - 1),
                    )
                nc.scalar.add(qtpair[:, ts(ch, 512)], ps, bqq_sb)

            emit_kv(0)
            emit_q(0)
            emit_q(1)

            # interleave remaining projections into pass 0's kp loop
            proj_sched = {2 * (ch - 1): ch for ch in range(1, NCH)}
            q_sched = {9: 2, 11: 3}

            def norm(zp, qc, act_zsb=True, act_mul=False):
                # evacuate Z'^T, transpose, normalize. Engine choice per
                # step so the two final-pass norms run concurrently on
                # ScalarE and VectorE (scalar.mul = Copy with scale AP).
                zsb = zsbp.tile([E + 1, 512], F32, tag="zsb", name=f"zsb{qc}")
                if act_zsb:
                    nc.scalar.copy(zsb, zp)
                else:
                    nc.vector.tensor_copy(zsb, zp)
                zt = psA.tile([128, 4, E + 1], F32, tag="mm", name=f"zt{qc}")
                for sub in range(4):
                    nc.tensor.transpose(
                        zt[:, sub, :], zsb[:, ts(sub, 128)], ident_f32
                    )
                o_sb = outp.tile([128, 4, E], F32, tag="o", name=f"o{qc}")
                for sub in range(4):
                    r = small.tile([128, 1], F32, tag="r", name=f"r{qc}_{sub}")
                    nc.vector.reciprocal(r, zt[:, sub, E:E + 1])
                    if act_mul:
                        nc.scalar.mul(o_sb[:, sub, :], zt[:, sub, 0:E], r)
                    else:
                        nc.vector.tensor_scalar_mul(
                            o_sb[:, sub, :], zt[:, sub, 0:E], r
                        )
                y_ap = y[ts(qc, 512), :].rearrange("(t p) e -> p t e", t=4)
                nc.gpsimd.dma_start(out=y_ap, in_=o_sb)

            def emit_av(zp0, zp1, kp, p0, p1):
                # AV matmuls grouped by stationary V' tile; zp row 64
                # accumulates the softmax denominator via the ones column.
                # On the final pair, finish zp0 first so its normalization
                # can begin two matmul slots earlier.
                p1b = p1.bitcast(BF16)
                ka, kb = 2 * kp, 2 * kp + 1
                st, fin = kp == 0, kp == KP - 1
                if fin:
                    nc.tensor.matmul(
                        zp0, vp_sb[:, ka, :], p0[:, 0:512],
                        start=st, stop=False,
                    )
                    nc.tensor.matmul(
                        zp0, vp_sb[:, kb, :], p0[:, 512:1024],
                        start=False, stop=True,
                    )
                    nc.tensor.matmul(
                        zp1, vp_sb[:, ka, :], p1b[:, 0:512],
                        start=st, stop=False,
                    )
                    nc.tensor.matmul(
                        zp1, vp_sb[:, kb, :], p1b[:, 512:1024],
                        start=False, stop=True,
                    )
                    return
                nc.tensor.matmul(
                    zp0, vp_sb[:, ka, :], p0[:, 0:512],
                    start=st, stop=False,
                )
                nc.tensor.matmul(
                    zp1, vp_sb[:, ka, :], p1b[:, 0:512],
                    start=st, stop=False,
                )
                nc.tensor.matmul(
                    zp0, vp_sb[:, kb, :], p0[:, 512:1024],
                    start=False, stop=False,
                )
                nc.tensor.matmul(
                    zp1, vp_sb[:, kb, :], p1b[:, 512:1024],
                    start=False, stop=False,
                )

            norm_sched = {}  # pass_ -> {kp: (zp, qc)}

            for pass_ in range(2):
                qa, qb = 2 * pass_, 2 * pass_ + 1
                zp0 = psZ.tile([E + 1, 512], F32, tag="z", name=f"zp{qa}")
                zp1 = psZ.tile([E + 1, 512], F32, tag="z", name=f"zp{qb}")
                pend = None
                for kp in range(KP):
                    if pass_ == 0:
                        if kp in proj_sched:
                            emit_kv(proj_sched[kp])
                        if kp in q_sched:
                            emit_q(q_sched[kp])
                    if kp in norm_sched.get(pass_, {}):
                        norm(*norm_sched[pass_][kp])
                    ka, kb = 2 * kp, 2 * kp + 1
                    sp0 = psS.tile(
                        [128, 1024], F32, tag="sp", name=f"sp{pass_}_{kp}a"
                    )
                    sp1 = psS.tile(
                        [128, 1024], F32, tag="sp", name=f"sp{pass_}_{kp}b"
                    )
                    # grouped by stationary operand (k-tile), row-group
                    # a/b pairs run concurrently on the PE
                    nc.tensor.matmul(
                        sp0[:, 0:512], krep[:, ts(ka, 128)],
                        qtpair[0:64, ts(qa, 512)], start=True, stop=True,
                    )
                    nc.tensor.matmul(
                        sp1[:, 0:512], krep[:, ts(ka, 128)],
                        qtpair[0:64, ts(qb, 512)], start=True, stop=True,
                    )
                    nc.tensor.matmul(
                        sp0[:, 512:1024], kv_sb[64:128, ts(kb, 128)],
                        qtpair[64:128, ts(qa, 512)], start=True, stop=True,
                    )
                    nc.tensor.matmul(
                        sp1[:, 512:1024], kv_sb[64:128, ts(kb, 128)],
                        qtpair[64:128, ts(qb, 512)], start=True, stop=True,
                    )
                    # exp: ScalarE exact for chunk A, VectorE bit-trick for B
                    p0 = pp0.tile(
                        [128, 1024], BF16, tag="p0", name=f"p{pass_}_{kp}a"
                    )
                    nc.scalar.activation(p0, sp0, EXP, scale=SCALE)
                    p1 = pp1.tile(
                        [128, 1024], I16, tag="p1", name=f"p{pass_}_{kp}b"
                    )
                    nc.vector.tensor_scalar(
                        p1, sp1, A_SCH, B_SCH, MULT, ADD
                    )
                    if pend is not None:
                        emit_av(zp0, zp1, *pend)
                    pend = (kp, p0, p1)
                emit_av(zp0, zp1, *pend)
                norm_sched[pass_ + 1] = {1: (zp0, qa), 3: (zp1, qb)}
            # final pass: run the two chunk norms concurrently on ACT + DVE
            (zpa, qca), (zpb, qcb) = (
                norm_sched[2][1], norm_sched[2][3]
            )
            norm(zpa, qca, act_zsb=True, act_mul=True)
            norm(zpb, qcb, act_zsb=False, act_mul=False)
    nc.compile()
    return nc


def get_nc():
    if "nc" not in _NC_CACHE:
        _NC_CACHE["nc"] = build_nc()
    return _NC_CACHE["nc"]


def make_in_maps(x, Wq, bq, Wk, bk, Wv, bv):
    x = np.asarray(x, dtype=np.float32)
    Wq = np.asarray(Wq, dtype=np.float32)
    Wk = np.asarray(Wk, dtype=np.float32)
    Wv = np.asarray(Wv, dtype=np.float32)
    bq = np.asarray(bq, dtype=np.float32)

    wkv_host = np.ascontiguousarray(
        np.concatenate([Wv.T, Wk.T], axis=1)
    ).astype(NP_BF16)
    wqq_host = np.ascontiguousarray(
        np.concatenate([Wq.T, Wq.T], axis=1)
    ).astype(NP_BF16)
    bqq_host = np.ascontiguousarray(
        np.concatenate([bq, bq]).reshape(128, 1)
    ).astype(np.float32)

    in_maps = []
    for c in range(8):
        b, h = c // 2, c % 2
        xT_b = np.asarray(x[b].T, dtype=NP_BF16)
        if h == 1:  # put this core's query half into columns [0, 2048)
            xT_b = np.concatenate([xT_b[:, TQ:], xT_b[:, :TQ]], axis=1)
        in_maps.append(
            {
                "xT": np.ascontiguousarray(xT_b),
                "wqq": wqq_host,
                "wkv": wkv_host,
                "bqq": bqq_host,
            }
        )
    return in_maps


def assemble(results, bv):
    bv = np.asarray(bv, dtype=np.float32)
    out = np.empty((B, S_FULL, E), dtype=np.float32)
    for c in range(8):
        b, h = c // 2, c % 2
        out[b, h * TQ:(h + 1) * TQ, :] = results[c]["y"] + bv
    return out


def kernel(x, Wq, bq, Wk, bk, Wv, bv, **_unused):
    in_maps = make_in_maps(x, Wq, bq, Wk, bk, Wv, bv)
    nc = get_nc()
    res = run_bass_kernel_spmd(nc, in_maps, core_ids=list(range(8)))
    return assemble(res.results, bv)


# revision 16
# speedup vs baseline: 1.2315x; 1.2315x over previous
"""Attention (B=4, S=4096, W=512, E=64) on 8 TRN2 NeuronCores.

Sharding: core c handles batch b = c//2, query half h = c%2 (2048 queries).
Each core receives x[b]^T as bf16 with the key/value columns ordered so that
this core's query half occupies columns [0, 2048) (softmax over keys is
permutation invariant as long as K and V share the order). K/V are computed
for the full sequence locally; flash-style attention over the core's query
half. No collectives.

Math simplifications vs the reference:
  - K bias bk drops out of softmax entirely (its contribution to the score
    is constant along the key axis only through Q.bk, a per-query constant).
  - V bias bv commutes with the softmax average: Z = softmax(S) V + bv, so
    bv is added on the host during unsharding.
  - Only the Q bias bq remains on-device (fused into the Q projection
    evacuation on ScalarE as an Identity-activation with per-partition bias).

Per-core dataflow (bf16 matmul inputs, fp32 PSUM):
  x^T [512,4096] streamed in 16 HWDGE transfers issued from 4 different
  engines (parallel descriptor generation); ~14 warmup matmuls run during
  the DMA wait so the PE HAM clock-gate is released before real work.
  Projections: kv = V^T (p0:64) / K^T (p64:128), Q^T duplicated on both
  halves; PSUM evacuation on ScalarE (frees VectorE for exp work).
  K^T replicated to partitions 0:64 via SBUF->SBUF DMA (score row-group 0).
  Attention in two passes over query-chunk pairs; per key-tile pair kp:
    scores S^T for two chunks, two k-tiles packed in PE row groups (e=64),
    matmuls grouped by stationary operand;
    exp: ScalarE does chunk A exactly (ACT table), VectorE does chunk B via
    a Schraudolph bit-trick (one tensor_scalar mul-add fp32->int16, the
    int16 buffer reinterpreted as bf16 ~= exp with ~3% max error, which
    softmax normalization cancels to ~1e-3 in Z);
    AV accumulated into per-chunk PSUM banks with a trailing ones column
    in V' producing the softmax denominator.
  Normalize: PE-transpose Z'^T, reciprocal + scale on VectorE, DMA out.
"""

import numpy as np
import ml_dtypes

import concourse.bass as bass
import concourse.mybir as mybir
import concourse.tile as tile
from concourse import bacc
from concourse.bass import ts
from concourse.masks import make_identity
from concourse.bass_utils import run_bass_kernel_spmd

BF16 = mybir.dt.bfloat16
F32 = mybir.dt.float32
I16 = mybir.dt.int16
F8 = mybir.dt.float8e4
I8 = mybir.dt.int8
NP_BF16 = ml_dtypes.bfloat16

B = 4
S_FULL = 4096
W = 512
E = 64
TQ = 2048  # queries per core
WT = W // 128  # 4 contraction tiles
KT = S_FULL // 128  # 32 key tiles
KP = KT // 2  # 16 key-tile pairs
NCH = S_FULL // 512  # 8 projection chunks
QC = TQ // 512  # 4 query chunks of 512
SCALE = 0.125  # 1/sqrt(E)
N_WARMUP = 14

# Schraudolph exp-approximation constants for bf16 bit-trick on VectorE:
# bits16(exp(SCALE*s)) ~= round(A*s + Bc); bf16 = 8 exp bits, 7 mantissa.
A_SCH = float(SCALE * np.log2(np.e) * 128.0)
B_SCH = float(127.0 * 128.0 - 5.8)
# fp8 e4m3 variant (DoubleRow AV path): bits8 ~= round(8*y + 56 - c)
A_SCH8 = float(SCALE * np.log2(np.e) * 8.0)
B_SCH8 = float(7.0 * 8.0 - 0.5)
MPAD = 80  # V' columns padded so the DoubleRow Ko step is 16B-aligned

_NC_CACHE = {}


def build_nc():
    nc = bacc.Bacc("TRN2", target_bir_lowering=False)
    xT = nc.dram_tensor("xT", [W, S_FULL], BF16, kind="ExternalInput")
    wqq = nc.dram_tensor("wqq", [W, 128], BF16, kind="ExternalInput")
    wkv = nc.dram_tensor("wkv", [W, 128], BF16, kind="ExternalInput")
    bqq = nc.dram_tensor("bqq", [128, 1], F32, kind="ExternalInput")
    y = nc.dram_tensor("y", [TQ, E], F32, kind="ExternalOutput")

    MULT = mybir.AluOpType.mult
    ADD = mybir.AluOpType.add
    EXP = mybir.ActivationFunctionType.Exp

    with tile.TileContext(nc) as tc:
        with (
            tc.tile_pool(name="const", bufs=1) as const,
            tc.tile_pool(name="psA", bufs=2, space="PSUM") as psA,
            tc.tile_pool(name="psS", bufs=2, space="PSUM") as psS,
            tc.tile_pool(name="psZ", bufs=2, space="PSUM") as psZ,
            tc.tile_pool(name="pp0", bufs=2) as pp0,
            tc.tile_pool(name="pp1", bufs=2) as pp1,
            tc.tile_pool(name="zsb", bufs=2) as zsbp,
            tc.tile_pool(name="small", bufs=2) as small,
            tc.tile_pool(name="outp", bufs=2) as outp,
        ):
            # weights/bias first so warmup matmuls can start early
            wkv_sb = const.tile([128, WT, 128], BF16)
            wqq_sb = const.tile([128, WT, 128], BF16)
            nc.scalar.dma_start(
                out=wkv_sb, in_=wkv[:, :].rearrange("(t p) m -> p t m", t=WT)
            )
            nc.scalar.dma_start(
                out=wqq_sb, in_=wqq[:, :].rearrange("(t p) m -> p t m", t=WT)
            )
            bqq_sb = const.tile([128, 1], F32)
            nc.scalar.dma_start(out=bqq_sb, in_=bqq[:, :])

            # x^T streamed in; descriptor generation spread over 4 engines
            xt_sb = const.tile([128, WT, S_FULL], BF16)
            dma_engines = [nc.sync, nc.scalar]
            for ch2 in range(NCH // 2):
                for t in range(WT):
                    eng = dma_engines[(ch2 * WT + t) % 2]
                    eng.dma_start(
                        out=xt_sb[:, t, ts(ch2, 1024)],
                        in_=xT[t * 128:(t + 1) * 128, ts(ch2, 1024)],
                    )

            ident_bf = const.tile([64, 64], BF16)
            make_identity(nc, ident_bf)
            ident_f32 = const.tile([E + 1, E + 1], F32)
            make_identity(nc, ident_f32)

            kv_sb = const.tile([128, S_FULL], BF16)  # V^T (p0:64) / K^T (p64:)
            krep = const.tile([64, S_FULL], BF16)  # K^T replica on p0:64
            qtpair = const.tile([128, TQ], BF16)  # Q^T on both halves
            # V' = [V | 1 | pad] in fp8, DoubleRow-interleaved per k-pair
            vp_sb = const.tile([128, KP, 2, MPAD], F8)
            nc.gpsimd.memset(vp_sb, 1.0)

            # warmup matmuls: junk results, keep PE busy through the HAM
            # cold window while the x stream is in flight
            wu = psA.tile([128, 512], F32, tag="mm", name="wu")
            for i in range(N_WARMUP):
                nc.tensor.matmul(
                    wu, wkv_sb[:, i % WT, :], wqq_sb[:, :, :],
                    start=True, stop=True,
                )

            def emit_kv(ch):
                ps = psA.tile([128, 512], F32, tag="mm", name=f"pskv{ch}")
                for t in range(WT):
                    nc.tensor.matmul(
                        ps,
                        wkv_sb[:, t, :],
                        xt_sb[:, t, ts(ch, 512)],
                        start=(t == 0),
                        stop=(t == WT - 1),
                    )
                nc.scalar.copy(kv_sb[:, ts(ch, 512)], ps)
                nc.sync.dma_start(
                    out=krep[:, ts(ch, 512)], in_=kv_sb[64:128, ts(ch, 512)]
                )
                # V^T -> V' tiles (4 PE transposes, one batched DVE copy)
                vt = psA.tile([128, 4, E], BF16, tag="mm", name=f"vt{ch}")
                for j in range(4):
                    nc.tensor.transpose(
                        vt[:, j, :], kv_sb[0:64, ts(4 * ch + j, 128)], ident_bf
                    )
                nc.vector.tensor_copy(
                    vp_sb[:, 2 * ch:2 * ch + 2, :, 0:E], vt
                )

            def emit_q(ch):
                ps = psA.tile([128, 512], F32, tag="mm", name=f"psq{ch}")
                for t in range(WT):
                    nc.tensor.matmul(
                        ps,
                        wqq_sb[:, t, :],
                        xt_sb[:, t, ts(ch, 512)],
                        start=(t == 0),
                        stop=(t == WT - 1),
                    )
                nc.scalar.add(qtpair[:, ts(ch, 512)], ps, bqq_sb)

            emit_kv(0)
            emit_q(0)
            emit_q(1)

            # interleave remaining projections into pass 0's kp loop
            proj_sched = {2 * (ch - 1): ch for ch in range(1, NCH)}
            q_sched = {9: 2, 11: 3}

            def norm(zp, qc, act_zsb=True, act_mul=False, out_sync=False):
                # evacuate Z'^T, transpose, normalize. Engine choice per
                # step so the two final-pass norms run concurrently on
                # ScalarE and VectorE (scalar.mul = Copy with scale AP).
                zsb = zsbp.tile([E + 1, 512], F32, tag="zsb", name=f"zsb{qc}")
                if act_zsb:
                    nc.scalar.copy(zsb, zp[0:E + 1, :])
                else:
                    nc.vector.tensor_copy(zsb, zp[0:E + 1, :])
                zt = psA.tile([128, 4, E + 1], F32, tag="mm", name=f"zt{qc}")
                for sub in range(4):
                    nc.tensor.transpose(
                        zt[:, sub, :], zsb[:, ts(sub, 128)], ident_f32
                    )
                o_sb = outp.tile([128, 4, E], F32, tag="o", name=f"o{qc}")
                r4 = small.tile([128, 4, 1], F32, tag="r", name=f"r{qc}")
                nc.vector.reciprocal(r4, zt[:, :, E:E + 1])
                for sub in range(4):
                    if act_mul:
                        nc.scalar.mul(
                            o_sb[:, sub, :], zt[:, sub, 0:E], r4[:, sub, 0:1]
                        )
                    else:
                        nc.vector.tensor_scalar_mul(
                            o_sb[:, sub, :], zt[:, sub, 0:E], r4[:, sub, 0:1]
                        )
                y_ap = y[ts(qc, 512), :].rearrange("(t p) e -> p t e", t=4)
                eng = nc.sync if out_sync else nc.gpsimd
                eng.dma_start(out=y_ap, in_=o_sb)

            DR = mybir.MatmulPerfMode.DoubleRow

            def emit_av(zp0, zp1, kp, p0, p1):
                # one fp8 DoubleRow matmul per (k-pair, chunk): contraction
                # 256 keys, V' ones column accumulates the denominator
                p1b = p1.bitcast(F8)
                st, fin = kp == 0, kp == KP - 1
                nc.tensor.matmul(
                    zp0, vp_sb[:, kp, :, :], p0,
                    start=st, stop=fin, perf_mode=DR,
                )
                nc.tensor.matmul(
                    zp1, vp_sb[:, kp, :, :], p1b,
                    start=st, stop=fin, perf_mode=DR,
                )

            norm_sched = {}  # pass_ -> {kp: (zp, qc)}

            for pass_ in range(2):
                qa, qb = 2 * pass_, 2 * pass_ + 1
                zp0 = psZ.tile([MPAD, 512], F32, tag="z", name=f"zp{qa}")
                zp1 = psZ.tile([MPAD, 512], F32, tag="z", name=f"zp{qb}")
                pend = None
                for kp in range(KP):
                    if pass_ == 0:
                        if kp in proj_sched:
                            emit_kv(proj_sched[kp])
                        if kp in q_sched:
                            emit_q(q_sched[kp])
                    if kp in norm_sched.get(pass_, {}):
                        norm(*norm_sched[pass_][kp])
                    ka, kb = 2 * kp, 2 * kp + 1
                    sp0 = psS.tile(
                        [128, 1024], F32, tag="sp", name=f"sp{pass_}_{kp}a"
                    )
                    sp1 = psS.tile(
                        [128, 1024], F32, tag="sp", name=f"sp{pass_}_{kp}b"
                    )
                    # grouped by stationary operand (k-tile), row-group
                    # a/b pairs run concurrently on the PE
                    nc.tensor.matmul(
                        sp0[:, 0:512], krep[:, ts(ka, 128)],
                        qtpair[0:64, ts(qa, 512)], start=True, stop=True,
                    )
                    nc.tensor.matmul(
                        sp1[:, 0:512], krep[:, ts(ka, 128)],
                        qtpair[0:64, ts(qb, 512)], start=True, stop=True,
                    )
                    nc.tensor.matmul(
                        sp0[:, 512:1024], kv_sb[64:128, ts(kb, 128)],
                        qtpair[64:128, ts(qa, 512)], start=True, stop=True,
                    )
                    nc.tensor.matmul(
                        sp1[:, 512:1024], kv_sb[64:128, ts(kb, 128)],
                        qtpair[64:128, ts(qb, 512)], start=True, stop=True,
                    )
                    # exp: ScalarE exact for chunk A, VectorE bit-trick for B
                    p0 = pp0.tile(
                        [128, 2, 512], F8, tag="p0", name=f"p{pass_}_{kp}a"
                    )
                    nc.scalar.activation(p0, sp0, EXP, scale=SCALE)
                    p1 = pp1.tile(
                        [128, 2, 512], I8, tag="p1", name=f"p{pass_}_{kp}b"
                    )
                    nc.vector.tensor_scalar(
                        p1, sp1, A_SCH8, B_SCH8, MULT, ADD
                    )
                    if pend is not None:
                        emit_av(zp0, zp1, *pend)
                    pend = (kp, p0, p1)
                emit_av(zp0, zp1, *pend)
                norm_sched[pass_ + 1] = {1: (zp0, qa), 3: (zp1, qb)}
            # final pass: run the two chunk norms concurrently on ACT + DVE
            (zpa, qca), (zpb, qcb) = (
                norm_sched[2][1], norm_sched[2][3]
            )
            norm(zpa, qca, act_zsb=True, act_mul=True, out_sync=True)
            norm(zpb, qcb, act_zsb=False, act_mul=False)
    nc.compile()
    return nc


def get_nc():
    if "nc" not in _NC_CACHE:
        _NC_CACHE["nc"] = build_nc()
    return _NC_CACHE["nc"]


def make_in_maps(x, Wq, bq, Wk, bk, Wv, bv):
    x = np.asarray(x, dtype=np.float32)
    Wq = np.asarray(Wq, dtype=np.float32)
    Wk = np.asarray(Wk, dtype=np.float32)
    Wv = np.asarray(Wv, dtype=np.float32)
    bq = np.asarray(bq, dtype=np.float32)

    wkv_host = np.ascontiguousarray(
        np.concatenate([Wv.T, Wk.T], axis=1)
    ).astype(NP_BF16)
    wqq_host = np.ascontiguousarray(
        np.concatenate([Wq.T, Wq.T], axis=1)
    ).astype(NP_BF16)
    bqq_host = np.ascontiguousarray(
        np.concatenate([bq, bq]).reshape(128, 1)
    ).astype(np.float32)

    in_maps = []
    for c in range(8):
        b, h = c // 2, c % 2
        xT_b = np.asarray(x[b].T, dtype=NP_BF16)
        if h == 1:  # put this core's query half into columns [0, 2048)
            xT_b = np.concatenate([xT_b[:, TQ:], xT_b[:, :TQ]], axis=1)
        in_maps.append(
            {
                "xT": np.ascontiguousarray(xT_b),
                "wqq": wqq_host,
                "wkv": wkv_host,
                "bqq": bqq_host,
            }
        )
    return in_maps


def assemble(results, bv):
    bv = np.asarray(bv, dtype=np.float32)
    out = np.empty((B, S_FULL, E), dtype=np.float32)
    for c in range(8):
        b, h = c // 2, c % 2
        out[b, h * TQ:(h + 1) * TQ, :] = results[c]["y"] + bv
    return out


def kernel(x, Wq, bq, Wk, bk, Wv, bv, **_unused):
    in_maps = make_in_maps(x, Wq, bq, Wk, bk, Wv, bv)
    nc = get_nc()
    res = run_bass_kernel_spmd(nc, in_maps, core_ids=list(range(8)))
    return assemble(res.results, bv)
